# revision 2
# baseline (speedup 1.0000x reference)
"""CfC (nn_Actor) Trainium2 kernel: 8-core data-parallel recurrent scan.

Layout strategy: everything "transposed" (features on SBUF partitions, batch on
the free dim) so the recurrent state hT feeds the next step's matmuls with zero
data movement. Host-side numpy does all the layout marshalling: x is
pre-transposed with a ones-row appended (folds the backbone bias into the
matmul), weights are pre-transposed/pre-scaled (lecun_tanh scales folded into
weights; sigmoid computed with the 0.5 folded in).

Two independent half-batch scans (ILV=2) are interleaved so one scan's matmuls
fill the other's activation/vector-engine bubbles (the step chain is serial).

v2: the two scans SHARE one [128,2048] PSUM tile per step (4 banks):
  bank0 = A.ff1|A.ff2, bank1 = A.tau|B.tau, bank2 = B.ff1|B.ff2,
  bank3 = A.gpre|B.gpre.
Biases enter PSUM via 3 bank-cleaner matmuls (K=8, N=512) placed in distinct
32-row PE groups (row 0/32/64) so they execute concurrently in the array, and
ONE joint x-matmul (N=128) covers both scans' backbone x-contribution —
replacing the 6 per-scan bias/x matmuls of v1.
"""

import numpy as np
import ml_dtypes

import concourse.bass as bass
import concourse.tile as tile
from concourse import bacc, mybir
from concourse.bass_utils import run_bass_kernel_spmd

BF16 = mybir.dt.bfloat16
F32 = mybir.dt.float32
TANH = mybir.ActivationFunctionType.Tanh
SIGMOID = mybir.ActivationFunctionType.Sigmoid
ALU = mybir.AluOpType

B, T, D, H, BBD, A = 1024, 256, 64, 512, 128, 8
NCORES = 8
BC = B // NCORES  # batch per core = 128
NK = H // 128  # 4 h-tiles
ILV = 2
W = BC // ILV  # 64

S_IN = 0.666
S_OUT = 1.7159

# psf column map (fp32 cols within the shared [128, 2048] tile)
#   A: ff1_k @ 64k, ff2_k @ 256+64k, tau_k @ 512+64k
#   B: tau_k @ 768+64k, ff1_k @ 1024+64k, ff2_k @ 1280+64k
#   gpre: A @ 1536:1600, B @ 1600:1664
FF1_OFF = (0, 1024)
FF2_OFF = (256, 1280)
TAU_OFF = (512, 768)
GP_OFF = (1536, 1600)

_CACHE = {}


def _emit_scan(nc, tc, pools, consts, s, psf_seq):
    """Emit one scan (batch slice s, width W) over T steps.

    psf_seq[t] is the shared PSUM tile for step t (allocated by the driver).
    """
    gpool, ffvpool, hpool, wpool, psf_pool = pools
    xT, wbbh, wbbx, wff, biasc, indc, wf1, bf1T, wf2, bf2, ones = consts

    h_prev = None
    for t in range(T):
        psf = psf_seq[t]
        gp = GP_OFF[s]
        if t > 0:
            for k in range(NK):
                nc.tensor.matmul(
                    psf[:, gp : gp + W],
                    wbbh[:, k * 128 : (k + 1) * 128],
                    h_prev[:, k * W : (k + 1) * W],
                    start=False,
                    stop=(s == 1 and k == NK - 1),
                    skip_group_check=True,
                )
        gT = gpool.tile([128, W], BF16, tag=f"gT{s}")
        gact = nc.scalar.activation(gT[:], psf[:, gp : gp + W], TANH)

        # ---- tau | ff1 | ff2 data matmuls (tau first: its sigmoid then
        # overlaps the remaining ff matmuls on the Scalar engine) ----
        for j, off in ((2, TAU_OFF[s]), (0, FF1_OFF[s]), (1, FF2_OFF[s])):
            for k in range(NK):
                c0 = off + k * W
                # last writer per bank: bank0=A.ff2 k3, bank1=B.tau k3,
                # bank2=B.ff2 k3
                is_stop = (j == 1 and k == NK - 1) or (
                    s == 1 and j == 2 and k == NK - 1
                )
                nc.tensor.matmul(
                    psf[:, c0 : c0 + W],
                    wff[:, (j * NK + k) * 128 : (j * NK + k + 1) * 128],
                    gT[:],
                    start=False,
                    stop=is_stop,
                    skip_group_check=True,
                )
        tsig = wpool.tile([128, 4 * W], BF16, tag=f"ts{s}")
        sig = nc.scalar.activation(
            tsig[:], psf[:, TAU_OFF[s] : TAU_OFF[s] + 4 * W], SIGMOID, scale=2.0
        )
        ffv = ffvpool.tile([128, 8 * W], BF16, tag=f"ffv{s}")
        bigact = nc.scalar.activation(
            ffv[:], psf[:, FF1_OFF[s] : FF1_OFF[s] + 8 * W], TANH
        )
        ff1v = ffv[:, 0 : 4 * W]
        ff2v = ffv[:, 4 * W : 8 * W]

        # ---- combine: h = ff1 + sig*(ff2-ff1) ----
        u = wpool.tile([128, 4 * W], BF16, tag=f"u{s}")
        uop = nc.vector.tensor_tensor(u[:], ff2v, ff1v, ALU.subtract)
        v = wpool.tile([128, 4 * W], BF16, tag=f"v{s}")
        nc.vector.tensor_tensor(v[:], u[:], tsig[:], ALU.mult)
        h = hpool.tile([128, 4 * W], BF16, tag=f"h{s}")
        hop = nc.vector.tensor_tensor(h[:], ff1v, v[:], ALU.add)
        h_prev = h
        yield (gact, sig, bigact, uop, hop)

    # ---- head: y1 = tanh(Wf1 @ h + bf1) ----
    psy = psf_pool.tile([128, 2048], F32, tag="psf")
    nc.tensor.matmul(
        psy[:, 0 : 4 * W], bf1T[0:4, :], indc[0:4, 0 : 4 * W],
        start=True, stop=False,
    )
    for m in range(4):
        for k in range(NK):
            nc.tensor.matmul(
                psy[:, m * W : (m + 1) * W],
                wf1[:, (m * NK + k) * 128 : (m * NK + k + 1) * 128],
                h_prev[:, k * W : (k + 1) * W],
                start=False, stop=(m == 3 and k == NK - 1),
            )
    y1v = ffvpool.tile([128, 4 * W], BF16, tag=f"ffv{s}")
    nc.scalar.activation(y1v[:], psy[:, 0 : 4 * W], TANH)

    # ---- head 2: out = tanh(Wf2 @ y1 + bf2) : [A=8, W] ----
    pso = psf_pool.tile([128, 2048], F32, tag="psf")
    nc.tensor.matmul(pso[0:A, 0:W], bf2[0:1, 0:A], ones[0:1, 0:W], start=True, stop=False)
    for k in range(NK):
        nc.tensor.matmul(
            pso[0:A, 0:W],
            wf2[:, k * A : (k + 1) * A],
            y1v[:, k * W : (k + 1) * W],
            start=False, stop=(k == NK - 1),
        )
    out_sb = wpool.tile([128, W], F32, tag=f"osb{s}")
    nc.scalar.activation(out_sb[0:A, :], pso[0:A, 0:W], TANH)
    return out_sb


def _build_program():
    nc = bacc.Bacc("TRN2", target_bir_lowering=False, debug=False)

    xT_e = nc.declare_dram_parameter("xT", [D + 1, T * BC], BF16, isOutput=False)
    wbbh_e = nc.declare_dram_parameter("wbbh", [128, NK * 128], BF16, isOutput=False)
    wbbx_e = nc.declare_dram_parameter("wbbx", [D + 1, 128], BF16, isOutput=False)
    wff_e = nc.declare_dram_parameter("wff", [128, 12 * 128], BF16, isOutput=False)
    biasc_e = nc.declare_dram_parameter("biasc", [72, 128], BF16, isOutput=False)
    indc_e = nc.declare_dram_parameter("indc", [72, 512], BF16, isOutput=False)
    wf1_e = nc.declare_dram_parameter("wf1", [128, 16 * 128], BF16, isOutput=False)
    bf1T_e = nc.declare_dram_parameter("bf1T", [4, 128], BF16, isOutput=False)
    wf2_e = nc.declare_dram_parameter("wf2", [128, NK * A], BF16, isOutput=False)
    bf2_e = nc.declare_dram_parameter("bf2", [1, A], BF16, isOutput=False)
    ones_e = nc.declare_dram_parameter("ones", [1, BC], BF16, isOutput=False)
    out_e = nc.declare_dram_parameter("out", [A, BC], F32, isOutput=True)

    from contextlib import ExitStack

    with tile.TileContext(nc) as tc, ExitStack() as ctx:
        const = ctx.enter_context(tc.tile_pool(name="const", bufs=1))
        gpool = ctx.enter_context(tc.tile_pool(name="g", bufs=4))
        ffvpool = ctx.enter_context(tc.tile_pool(name="ffv", bufs=4))
        hpool = ctx.enter_context(tc.tile_pool(name="h", bufs=4))
        wpool = ctx.enter_context(tc.tile_pool(name="work", bufs=4))
        psf_pool = ctx.enter_context(tc.tile_pool(name="psf", bufs=2, space="PSUM"))

        def cload(ext, shape, tag, dtype=BF16):
            t = const.tile(shape, dtype, tag=tag)
            nc.sync.dma_start(t[:], ext[:])
            return t

        consts = (
            cload(xT_e, [D + 1, T * BC], "xT"),
            cload(wbbh_e, [128, NK * 128], "wbbh"),
            cload(wbbx_e, [D + 1, 128], "wbbx"),
            cload(wff_e, [128, 12 * 128], "wff"),
            cload(biasc_e, [72, 128], "biasc"),
            cload(indc_e, [72, 512], "indc"),
            cload(wf1_e, [128, 16 * 128], "wf1"),
            cload(bf1T_e, [4, 128], "bf1T"),
            cload(wf2_e, [128, NK * A], "wf2"),
            cload(bf2_e, [1, A], "bf2"),
            cload(ones_e, [1, BC], "ones"),
        )
        pools = (gpool, ffvpool, hpool, wpool, psf_pool)
        xT_c, wbbx_c = consts[0], consts[2]
        biasc_c, indc_c = consts[4], consts[5]

        # Per-step shared PSUM prep: 3 concurrent bank cleaners + joint x-mm.
        def emit_step_prep(t):
            psf = psf_pool.tile([128, 2048], F32, tag="psf")
            cls = []
            for b, r0 in ((0, 0), (1, 32), (2, 64)):
                c = nc.tensor.matmul(
                    psf[:, b * 512 : (b + 1) * 512],
                    biasc_c[r0 : r0 + 8, :],
                    indc_c[r0 : r0 + 8, :],
                    start=True, stop=False, skip_group_check=True,
                )
                cls.append(c)
            xmm = nc.tensor.matmul(
                psf[:, 1536 : 1536 + BC],
                wbbx_c[:],
                xT_c[:, t * BC : (t + 1) * BC],
                start=True, stop=(t == 0), skip_group_check=True,
            )
            return psf, cls, xmm

        # Pre-allocate nothing: drive both scans step-interleaved, creating
        # the shared psf per step and handing it to both generators.
        psf_holder = [None]

        class PsfSeq:
            def __getitem__(self, t):
                return psf_holder[0]

        gens = [
            _emit_scan(nc, tc, pools, consts, s, PsfSeq()) for s in range(ILV)
        ]
        outs = [None] * ILV
        done = 0
        t = 0
        while done < ILV:
            if t < T:
                psf, cls, xmm = emit_step_prep(t)
                psf_holder[0] = psf
            for s, g in enumerate(gens):
                if outs[s] is None:
                    try:
                        next(g)
                    except StopIteration as e:
                        outs[s] = e.value
                        done += 1
            t += 1
        for s, osb in enumerate(outs):
            nc.sync.dma_start(out_e[:, s * W : (s + 1) * W], osb[0:A, :])

    nc.compile()
    return nc


def _to_bf16(x):
    return np.ascontiguousarray(x.astype(ml_dtypes.bfloat16))


def _prep_shared(Wbb, bbb, Wff1, bff1, Wff2, bff2, Wta, bta, Wtb, btb, Wf1, bf1, Wf2, bf2):
    Wbbx = Wbb[:, :D]  # [128, 64]
    Wbbh = Wbb[:, D:]  # [128, 512]

    wbbx_aug = np.zeros((D + 1, 128), np.float32)
    wbbx_aug[:D, :] = (S_IN * Wbbx).T
    wbbx_aug[D, :] = S_IN * bbb

    wbbh_arr = np.zeros((128, NK * 128), np.float32)
    for k in range(NK):
        wbbh_arr[:, k * 128 : (k + 1) * 128] = (S_IN * Wbbh[:, k * 128 : (k + 1) * 128]).T

    Aj = [S_OUT * Wff1, S_OUT * Wff2, 0.5 * S_OUT * (Wta + Wtb)]  # each [512, 128]
    bj = [bff1, bff2, 0.5 * (bta + btb)]
    wff_arr = np.zeros((128, 12 * 128), np.float32)
    for j in range(3):
        for k in range(NK):
            wff_arr[:, (j * NK + k) * 128 : (j * NK + k + 1) * 128] = Aj[j][
                k * 128 : (k + 1) * 128, :
            ].T

    # bias pack for the 3 shared bank cleaners (K=8 rows each, at PE row
    # groups 0/32/64). Bank images follow the psf column map:
    #   bank0 = A.ff1 | A.ff2, bank1 = A.tau | B.tau, bank2 = B.ff1 | B.ff2
    biasc_arr = np.zeros((72, 128), np.float32)
    biasc_arr[0:4, :] = bj[0].reshape(4, 128)
    biasc_arr[4:8, :] = bj[1].reshape(4, 128)
    biasc_arr[32:36, :] = bj[2].reshape(4, 128)
    biasc_arr[36:40, :] = bj[2].reshape(4, 128)
    biasc_arr[64:68, :] = bj[0].reshape(4, 128)
    biasc_arr[68:72, :] = bj[1].reshape(4, 128)

    # indicator: rows r select 64-col block r of the 512-col bank.
    # Rows 0-3 double as the y1-head bias indicator (blocks of W=64).
    indc_arr = np.zeros((72, 512), np.float32)
    for r0 in (0, 32, 64):
        for i in range(8):
            indc_arr[r0 + i, i * 64 : (i + 1) * 64] = 1.0

    wf1_arr = np.zeros((128, 16 * 128), np.float32)
    for m in range(4):
        for k in range(NK):
            wf1_arr[:, (m * NK + k) * 128 : (m * NK + k + 1) * 128] = Wf1[
                m * 128 : (m + 1) * 128, k * 128 : (k + 1) * 128
            ].T
    bf1T_arr = bf1.reshape(4, 128)

    wf2_arr = np.zeros((128, NK * A), np.float32)
    for k in range(NK):
        wf2_arr[:, k * A : (k + 1) * A] = Wf2[:, k * 128 : (k + 1) * 128].T
    bf2_arr = bf2.reshape(1, A)
    ones_arr = np.ones((1, BC), np.float32)

    return {
        "wbbh": _to_bf16(wbbh_arr),
        "wbbx": _to_bf16(wbbx_aug),
        "wff": _to_bf16(wff_arr),
        "biasc": _to_bf16(biasc_arr),
        "indc": _to_bf16(indc_arr),
        "wf1": _to_bf16(wf1_arr),
        "bf1T": _to_bf16(bf1T_arr),
        "wf2": _to_bf16(wf2_arr),
        "bf2": _to_bf16(bf2_arr),
        "ones": _to_bf16(ones_arr),
    }


def _prep_core_x(state_c):
    # state_c: [BC, T, D] -> xT_aug [D+1, T*BC] with ones row
    xT = state_c.transpose(2, 1, 0).reshape(D, T * BC)  # [d, t*BC+b]
    xa = np.ones((D + 1, T * BC), np.float32)
    xa[:D, :] = xT
    return _to_bf16(xa)


def _get_program():
    if "nc" not in _CACHE:
        _CACHE["nc"] = _build_program()
    return _CACHE["nc"]


def run(inputs, trace=False, trace_kwargs=None):
    inputs = {k: np.asarray(v) for k, v in inputs.items()}
    nc = _get_program()
    shared = _prep_shared(
        inputs["Wbb"], inputs["bbb"], inputs["Wff1"], inputs["bff1"],
        inputs["Wff2"], inputs["bff2"], inputs["Wta"], inputs["bta"],
        inputs["Wtb"], inputs["btb"], inputs["Wf1"], inputs["bf1"],
        inputs["Wf2"], inputs["bf2"],
    )
    state = inputs["state"].astype(np.float32)
    in_maps = []
    for c in range(NCORES):
        m = dict(shared)
        m["xT"] = _prep_core_x(state[c * BC : (c + 1) * BC])
        in_maps.append(m)
    res = run_bass_kernel_spmd(
        nc, in_maps, core_ids=list(range(NCORES)), trace=trace,
        **(trace_kwargs or {}),
    )
    out = np.concatenate(
        [np.asarray(res.results[c]["out"]).T for c in range(NCORES)], axis=0
    )
    return out.astype(np.float32), res


def kernel(**inputs):
    out, _ = run(inputs, trace=False)
    return out
